# revision 9
# baseline (speedup 1.0000x reference)
"""Trainium2 Bass kernel: GQA causal self-attention block (B=1, T=2048, D=2048,
32 q-heads / 8 kv-heads, head_dim 64) with q/k/v/o projections.

Sharding: head-parallel (tensor parallel) across 8 NeuronCores.
Core c owns q-heads 4c..4c+3 and kv-head c:
  - computes Q^T/K^T (transposed, head-dim on partitions) and V (natural) for
    its heads from a host-pre-transposed x^T,
  - blockwise causal softmax(QK^T)V in a k-major layout (denominator obtained
    free via a ones-column appended to V),
  - a partial output projection out_c = ctx_c^T @ o_proj[rows_c, :].
The host sums the 8 partial outputs (the tensor-parallel reduction).

All activations/weights are fp16 (full PE rate at any moving size, so causal
blocks are trimmed to live columns; PSUM accumulation stays fp32). DMAs are
batched into few large instructions; output staging copies alternate between
the DVE and Pool engines and output DMAs ride the Pool SWDGE queue, which
avoids the shared HWDGE serialization.
"""

import os
import numpy as np

T = 2048
D = 2048
HQ, HKV = 32, 8
DH = 64
NCORES = 8
PAIRS = 2                 # 2 head-pairs per core (4 q heads)
NCH = D // 128            # 16 contraction chunks for projections
NTQ = 4                   # t-quarters in projection phase
TQW = T // NTQ            # 512
NQB = 4                   # q blocks of 512
QBW = 512
NKB = T // 128            # 16 k blocks of 128
WCOLS = 4 * DH + 2 * DH   # 384: [q^T (256) | v^T (64) | k^T (64)]

_NC = None
LAST_RESULT = None


def build_nc():
    import concourse.tile as tile
    from concourse import bacc, mybir
    from concourse.masks import make_identity, make_upper_triangular

    f16 = mybir.dt.float16
    f32 = mybir.dt.float32
    Exp = mybir.ActivationFunctionType.Exp

    nc = bacc.Bacc("TRN2", target_bir_lowering=False, debug=False,
                   num_devices=NCORES)

    xt = nc.dram_tensor("xt", [D, T], f16, kind="ExternalInput").ap()
    wpt = nc.dram_tensor("wpt", [D, WCOLS], f16, kind="ExternalInput").ap()
    opj = nc.dram_tensor("opj", [4 * DH, D], f16, kind="ExternalInput").ap()
    out = nc.dram_tensor("out", [T, D], f16, kind="ExternalOutput").ap()

    from contextlib import ExitStack
    with tile.TileContext(nc) as tc, ExitStack() as ctx:
        consts = ctx.enter_context(tc.tile_pool(name="consts", bufs=1))
        wpool = ctx.enter_context(tc.tile_pool(name="weights", bufs=1))
        qtp = ctx.enter_context(tc.tile_pool(name="qt", bufs=1))
        ktp = ctx.enter_context(tc.tile_pool(name="kt", bufs=1))
        vpool = ctx.enter_context(tc.tile_pool(name="v", bufs=1))
        xpool = ctx.enter_context(tc.tile_pool(name="xchunk", bufs=4))
        epool = ctx.enter_context(tc.tile_pool(name="exps", bufs=18))
        cpool = ctx.enter_context(tc.tile_pool(name="ctxsb", bufs=1))
        spool = ctx.enter_context(tc.tile_pool(name="stage", bufs=2))
        opool = ctx.enter_context(tc.tile_pool(name="outsb", bufs=3))
        rpool = ctx.enter_context(tc.tile_pool(name="recip", bufs=2))
        # attention ST psum at stack bottom (banks 0-3) so phase B's ST/exp
        # can overlap phase A (whose psum lives in banks 4-7)
        stp = ctx.enter_context(tc.tile_pool(name="st_ps", bufs=2,
                                             space="PSUM"))

        # constants (built in f32 -- memset/affine_select write f32 -- then
        # converted to f16 via tensor_copy)
        identf = consts.tile([128, 128], f32, tag="identf")
        make_identity(nc, identf)
        ident = consts.tile([128, 128], f16, tag="ident")
        nc.vector.tensor_copy(out=ident, in_=identf)
        # mask[i, j] = 1.0 if i <= j else 0  (keep k_row <= q_col)
        maskf = consts.tile([128, 128], f32, tag="maskf")
        make_upper_triangular(nc, maskf, val=1.0, diag=True)
        mask = consts.tile([128, 128], f16, tag="mask")
        nc.vector.tensor_copy(out=mask, in_=maskf)
        onescf = consts.tile([128, NKB], f32, tag="onescf")
        nc.vector.memset(onescf, 1.0)
        onesc = consts.tile([128, NKB], f16, tag="onesc")
        nc.vector.tensor_copy(out=onesc, in_=onescf)
        onesrf = consts.tile([65, 64], f32, tag="onesrf")
        nc.vector.memset(onesrf, 1.0)
        onesr = consts.tile([65, 64], f16, tag="onesr")
        nc.vector.tensor_copy(out=onesr, in_=onesrf)

        # weights -> SBUF (two DMAs; first half unblocks the first matmuls)
        wpt_r = wpt.rearrange("(c p) n -> p c n", p=128)
        w_sb = wpool.tile([128, NCH, WCOLS], f16, tag="w")
        opj_r = opj.rearrange("(p r) j -> r p j", p=2)
        opj_sb = wpool.tile([128, 2, D], f16, tag="opj")

        # activation storage
        # qt_sb[p]: rows 0-63 = head 2p (Q^T), rows 64-127 = head 2p+1
        qt_sb = [qtp.tile([128, T], f16, tag=f"qt{p}", name=f"qt{p}")
                 for p in range(PAIRS)]
        # kv_sb: rows 0-63 = V^T, rows 64-127 = K^T
        kv_sb = ktp.tile([128, T], f16, tag="kv")
        # K^T copy on partitions 0-63 (for the tile_position (0,0) ST matmul)
        kt_a = ktp.tile([64, T], f16, tag="kta")
        # V natural [k, dh] per k-block, with a ones column at dh (denominator)
        v_sb = vpool.tile([128, NKB, DH + 1], f16, tag="vsb")
        nc.vector.tensor_copy(out=v_sb[:, :, DH], in_=onesc)
        # stacked normalized ctx^T: rows 0-63 head 2p, 64-127 head 2p+1;
        # axis 1 is the pair index
        ctx_sb = cpool.tile([128, PAIRS, T], f16, tag="ctx3", name="ctx3")

        # ---------------- helpers for interleaved emission ----------------
        pending_ex = {}

        def emit_stexp(qb, p, kb):
            q0 = QBW * qb
            kb_off = max(0, 128 * kb - q0)
            ex = epool.tile([128, 1024], f16, tag="ex", name="ex")
            for h in range(2):
                o = 512 * h
                st = stp.tile([128, 512], f32, tag="st", name="st")
                nc.tensor.matmul(
                    st[:, kb_off:512],
                    lhsT=(kt_a[:, 128 * kb:128 * kb + 128] if h == 0 else
                          kv_sb[64:128, 128 * kb:128 * kb + 128]),
                    rhs=qt_sb[p][64 * h:64 * h + 64,
                                 q0 + kb_off:q0 + QBW],
                    start=True, stop=True, tile_position=(64 * h, 0))
                nc.scalar.activation(
                    out=ex[:, o + kb_off:o + 512],
                    in_=st[:, kb_off:512], func=Exp)
            if 128 * kb >= q0:  # diagonal block: causal mask (both heads
                # in one 3D-AP multiply; mask broadcast along the head dim)
                ex3m = ex.rearrange("p (h q) -> p h q", h=2)
                nc.vector.tensor_mul(
                    ex3m[:, :, kb_off:kb_off + 128],
                    ex3m[:, :, kb_off:kb_off + 128],
                    mask.rearrange("p (h w) -> p h w", h=1).to_broadcast(
                        [128, 2, 128]))
            return ex

        # ---------------- Phase A: projections (t-quarters) ----------------
        with tc.tile_pool(name="pa_ps", bufs=1, space="PSUM") as pa:
            for tq in range(NTQ):
                t0 = TQW * tq
                qt_ps = [pa.tile([128, TQW], f32, tag=f"qtps{m}",
                                 name=f"qtps{m}") for m in range(2)]
                kv_ps = pa.tile([128, TQW], f32, tag="kvps")
                for grp in range(4):
                    if tq == 0 and grp == 0:
                        nc.sync.dma_start(out=w_sb[:, 0:4], in_=wpt_r[:, 0:4])
                    if tq == 0 and grp == 1:
                        nc.sync.dma_start(out=w_sb[:, 4:16],
                                          in_=wpt_r[:, 4:16])
                    xc = xpool.tile([128, 4, TQW], f16, tag="xc")
                    nc.sync.dma_start(
                        out=xc,
                        in_=xt.rearrange("(c p) n -> p c n", p=128)[
                            :, 4 * grp:4 * grp + 4, t0:t0 + TQW])
                    for ci4 in range(4):
                        ci = 4 * grp + ci4
                        for m in range(2):
                            nc.tensor.matmul(
                                qt_ps[m],
                                lhsT=w_sb[:, ci, 128 * m:128 * m + 128],
                                rhs=xc[:, ci4],
                                start=(ci == 0), stop=(ci == NCH - 1))
                        nc.tensor.matmul(
                            kv_ps, lhsT=w_sb[:, ci, 256:384], rhs=xc[:, ci4],
                            start=(ci == 0), stop=(ci == NCH - 1))
                for m in range(2):
                    nc.vector.tensor_copy(out=qt_sb[m][:, t0:t0 + TQW],
                                          in_=qt_ps[m])
                nc.vector.tensor_copy(out=kv_sb[:, t0:t0 + TQW], in_=kv_ps)
                # K^T duplicate for this quarter (cross-partition SBUF DMA)
                nc.sync.dma_start(out=kt_a[:, t0:t0 + TQW],
                                  in_=kv_sb[64:128, t0:t0 + TQW])
                # V natural via PE transpose of this quarter's V^T blocks
                for c in range(4 * tq, 4 * tq + 4):
                    tp = pa.tile([128, 64], f16, tag="vtr", name="vtr")
                    nc.tensor.transpose(
                        tp, in_=kv_sb[0:64, 128 * c:128 * c + 128],
                        identity=ident[0:64, 0:64])
                    nc.vector.tensor_copy(out=v_sb[:, c, 0:DH], in_=tp)
                # pre-emit attention ST/exp (runs in A's PE/ACT gaps)
                if tq == 0:
                    for p in range(PAIRS):
                        for kb in range(4):
                            pending_ex[(0, p, kb)] = emit_stexp(0, p, kb)
                elif tq == 1:
                    for kb in range(4):
                        pending_ex[(1, 0, kb)] = emit_stexp(1, 0, kb)
                elif tq == 2:
                    for kb in range(4):
                        pending_ex[(1, 1, kb)] = emit_stexp(1, 1, kb)

        # o_proj weights (first needed by phase C)
        nc.sync.dma_start(out=opj_sb[:, 0], in_=opj_r[:, 0])
        nc.sync.dma_start(out=opj_sb[:, 1], in_=opj_r[:, 1])

        # ---------------- Phase B (attention) + C (o_proj) ----------------
        with tc.tile_pool(name="ctx_ps", bufs=2, space="PSUM") as cxp, \
             tc.tile_pool(name="oc_ps", bufs=2, space="PSUM") as ocp:
            # C-unit state: emit o_proj tiles of the previous qb in drips
            cstate = {"units": [], "osb": None, "tt": -1}

            def emit_cunit():
                if not cstate["units"]:
                    return
                tt, jn = cstate["units"].pop(0)
                if cstate["tt"] != tt:
                    cstate["osb"] = opool.tile([128, D], f16, tag="osb",
                                               name="osb")
                    cstate["tt"] = tt
                osb = cstate["osb"]
                oc = ocp.tile([128, 512], f32, tag="oc", name="oc")
                for p in range(PAIRS):
                    nc.tensor.matmul(
                        oc,
                        lhsT=ctx_sb[:, p, 128 * tt:128 * tt + 128],
                        rhs=opj_sb[:, p, 512 * jn:512 * jn + 512],
                        start=(p == 0), stop=(p == PAIRS - 1))
                nc.vector.tensor_copy(out=osb[:, 512 * jn:512 * jn + 512],
                                      in_=oc)
                if jn == 3:  # whole row block staged -> one DMA (Pool SWDGE)
                    nc.gpsimd.dma_start(
                        out=out[128 * tt:128 * tt + 128, :], in_=osb)

            for qb in range(NQB):
                q0 = QBW * qb
                nkb = 4 * qb + 4
                stg = spool.tile([64, PAIRS, 512], f16, tag="stg",
                                 name="stg")
                for p in range(PAIRS):
                    ctx = cxp.tile([DH + 1, 1024], f32, tag="ctx")
                    for kb in range(nkb):
                        ex = pending_ex.pop((qb, p, kb), None)
                        if ex is None:
                            ex = emit_stexp(qb, p, kb)
                        # ctx^T (+ denominator row 64) accumulation; on
                        # diagonal blocks only cols >= kb_off are live
                        n0 = max(0, 128 * kb - q0)
                        for h in range(2):
                            o = 512 * h
                            nc.tensor.matmul(
                                ctx[:, o + n0:o + 512],
                                lhsT=v_sb[:, kb, :],
                                rhs=ex[:, o + n0:o + 512],
                                start=(kb == 0), stop=(kb == nkb - 1))
                        emit_cunit()
                    # denominator row (both heads) -> SBUF (f16) in one copy
                    densr = rpool.tile([65, 1024], f16, tag="densr")
                    nc.vector.tensor_copy(
                        out=densr[64:65, :], in_=ctx[64:65, :])
                    for h in range(2):
                        o = 512 * h
                        # replicate down 64 partitions with a K=1 matmul,
                        # then reciprocal
                        repl_ps = ocp.tile([64, 512], f32, tag="oc",
                                           name="replps")
                        nc.tensor.matmul(
                            repl_ps, lhsT=onesr[64:65, 0:64],
                            rhs=densr[64:65, o:o + 512],
                            start=True, stop=True, tile_position=(64, 0))
                        repl = rpool.tile([64, 512], f32, tag="repl")
                        nc.vector.reciprocal(out=repl, in_=repl_ps)
                        if h == 0:
                            nc.vector.tensor_mul(
                                ctx_sb[0:64, p, q0:q0 + QBW],
                                ctx[0:64, o:o + 512], repl)
                        else:
                            nc.vector.tensor_mul(
                                stg[:, p], ctx[0:64, o:o + 512], repl)
                # heads 2p+1 -> partitions 64-127 (cross-partition DMA),
                # both pairs in one transfer
                nc.sync.dma_start(out=ctx_sb[64:128, :, q0:q0 + QBW],
                                  in_=stg)
                # queue this qb's o_proj tiles; drain leftovers of qb-1 now
                while cstate["units"]:
                    emit_cunit()
                cstate["units"] = [(tt, jn) for tt in range(4 * qb, 4 * qb + 4)
                                   for jn in range(4)]
            while cstate["units"]:
                emit_cunit()

    nc.compile()
    return nc


def _get_nc():
    global _NC
    if _NC is None:
        _NC = build_nc()
    return _NC


def make_in_maps(x, q_proj, k_proj, v_proj, o_proj):
    x = np.asarray(x, np.float32).reshape(T, D)
    q_proj = np.asarray(q_proj, np.float32)
    k_proj = np.asarray(k_proj, np.float32)
    v_proj = np.asarray(v_proj, np.float32)
    o_proj = np.asarray(o_proj, np.float32)

    xt = np.ascontiguousarray(x.T).astype(np.float16)  # [D, T]
    scale = 1.0 / np.sqrt(np.float32(DH))
    maps = []
    for c in range(NCORES):
        qs = slice(4 * DH * c, 4 * DH * (c + 1))     # 256 q rows
        ks = slice(DH * c, DH * (c + 1))             # 64 kv rows
        m = {
            "xt": xt,
            # [q^T | v^T | k^T]: 384 columns
            "wpt": np.ascontiguousarray(np.concatenate(
                [q_proj[qs, :] * scale, v_proj[ks, :], k_proj[ks, :]],
                axis=0).T).astype(np.float16),
            "opj": np.ascontiguousarray(o_proj[qs, :]).astype(np.float16),
        }
        maps.append(m)
    return maps


def kernel(**inputs):
    global LAST_RESULT
    from concourse.bass_utils import run_bass_kernel_spmd
    nc = _get_nc()
    maps = make_in_maps(inputs["x"], inputs["q_proj"], inputs["k_proj"],
                        inputs["v_proj"], inputs["o_proj"])
    res = run_bass_kernel_spmd(
        nc, maps, list(range(NCORES)),
        trace=bool(int(os.environ.get("BASS_KERNEL_TRACE", "0"))))
    LAST_RESULT = res
    acc = np.zeros((T, D), np.float64)
    for c in range(NCORES):
        acc += res.results[c]["out"].astype(np.float64)
    return acc.astype(np.float32).reshape(1, T, D)


# revision 27
# speedup vs baseline: 1.2809x; 1.2809x over previous
"""Trainium2 Bass kernel: GQA causal self-attention block (B=1, T=2048, D=2048,
32 q-heads / 8 kv-heads, head_dim 64) with q/k/v/o projections.

Sharding: head-parallel (tensor parallel) across 8 NeuronCores.
Core c owns q-heads 4c..4c+3 and kv-head c:
  - computes Q^T/K^T (transposed, head-dim on partitions) and V (natural) for
    its heads from a host-pre-transposed x^T,
  - blockwise causal softmax(QK^T)V in a k-major layout (denominator obtained
    free via a ones-column appended to V),
  - a partial output projection out_c = ctx_c^T @ o_proj[rows_c, :].
The host sums the 8 partial outputs (the tensor-parallel reduction).

All activations/weights are fp16 (full PE rate at any moving size, so causal
blocks are trimmed to live columns; PSUM accumulation stays fp32). DMAs are
batched into few large instructions; output staging copies alternate between
the DVE and Pool engines and output DMAs ride the Pool SWDGE queue, which
avoids the shared HWDGE serialization.
"""

import os
import numpy as np

T = 2048
D = 2048
HQ, HKV = 32, 8
DH = 64
NCORES = 8
PAIRS = 2                 # 2 head-pairs per core (4 q heads)
NCH = D // 128            # 16 contraction chunks for projections
NTQ = 4                   # t-quarters in projection phase
TQW = T // NTQ            # 512
NQB = 4                   # q blocks of 512
QBW = 512
NKB = T // 128            # 16 k blocks of 128
WCOLS = 4 * DH + 2 * DH   # 384: [q^T (256) | v^T (64) | k^T (64)]

_NC = None
LAST_RESULT = None


def build_nc():
    import concourse.tile as tile
    from concourse import bacc, mybir
    from concourse.masks import make_identity, make_upper_triangular

    f16 = mybir.dt.float16
    f32 = mybir.dt.float32
    Exp = mybir.ActivationFunctionType.Exp
    Copy = mybir.ActivationFunctionType.Copy

    nc = bacc.Bacc("TRN2", target_bir_lowering=False, debug=False,
                   num_devices=NCORES)

    xt = nc.dram_tensor("xt", [D, T], f16, kind="ExternalInput").ap()
    wpt = nc.dram_tensor("wpt", [D, WCOLS], f16, kind="ExternalInput").ap()
    opj = nc.dram_tensor("opj", [4 * DH, D], f16, kind="ExternalInput").ap()
    out = nc.dram_tensor("out", [T, D], f16, kind="ExternalOutput").ap()

    from contextlib import ExitStack
    with tile.TileContext(nc) as tc, ExitStack() as ctx:
        consts = ctx.enter_context(tc.tile_pool(name="consts", bufs=1))
        wpool = ctx.enter_context(tc.tile_pool(name="weights", bufs=1))
        qtp = ctx.enter_context(tc.tile_pool(name="qt", bufs=1))
        ktp = ctx.enter_context(tc.tile_pool(name="kt", bufs=1))
        vpool = ctx.enter_context(tc.tile_pool(name="v", bufs=1))
        xpool = ctx.enter_context(tc.tile_pool(name="xchunk", bufs=4))
        epool = ctx.enter_context(tc.tile_pool(name="exps", bufs=18))
        cpool = ctx.enter_context(tc.tile_pool(name="ctxsb", bufs=1))
        spool = ctx.enter_context(tc.tile_pool(name="stage", bufs=2))
        opool = ctx.enter_context(tc.tile_pool(name="outsb", bufs=3))
        rpool = ctx.enter_context(tc.tile_pool(name="recip", bufs=2))
        # attention ST psum at stack bottom (banks 0-3) so phase B's ST/exp
        # can overlap phase A (whose psum lives in banks 4-7)
        stp = ctx.enter_context(tc.tile_pool(name="st_ps", bufs=2,
                                             space="PSUM"))

        # constants (built in f32 -- memset/affine_select write f32 -- then
        # converted to f16 via tensor_copy)
        identf = consts.tile([128, 128], f32, tag="identf")
        make_identity(nc, identf)
        ident = consts.tile([128, 128], f16, tag="ident")
        nc.vector.tensor_copy(out=ident, in_=identf)
        # mask[i, j] = 1.0 if i <= j else 0  (keep k_row <= q_col)
        maskf = consts.tile([128, 128], f32, tag="maskf")
        make_upper_triangular(nc, maskf, val=1.0, diag=True)
        mask = consts.tile([128, 128], f16, tag="mask")
        nc.vector.tensor_copy(out=mask, in_=maskf)
        onescf = consts.tile([128, NKB], f32, tag="onescf")
        nc.vector.memset(onescf, 1.0)
        onesc = consts.tile([128, NKB], f16, tag="onesc")
        nc.vector.tensor_copy(out=onesc, in_=onescf)
        onesrf = consts.tile([65, 64], f32, tag="onesrf")
        nc.vector.memset(onesrf, 1.0)
        onesr = consts.tile([65, 64], f16, tag="onesr")
        nc.vector.tensor_copy(out=onesr, in_=onesrf)

        # weights -> SBUF (two DMAs; first half unblocks the first matmuls)
        wpt_r = wpt.rearrange("(c p) n -> p c n", p=128)
        w_sb = wpool.tile([128, NCH, WCOLS], f16, tag="w")
        opj_r = opj.rearrange("(p r) j -> r p j", p=2)
        opj_sb = wpool.tile([128, 2, D], f16, tag="opj")

        # activation storage
        # qt_sb[p]: rows 0-63 = head 2p (Q^T), rows 64-127 = head 2p+1
        qt_sb = [qtp.tile([128, T], f16, tag=f"qt{p}", name=f"qt{p}")
                 for p in range(PAIRS)]
        # kv_sb: rows 0-63 = V^T, rows 64-127 = K^T
        kv_sb = ktp.tile([128, T], f16, tag="kv")
        # K^T copy on partitions 0-63 (for the tile_position (0,0) ST matmul)
        kt_a = ktp.tile([64, T], f16, tag="kta")
        # V natural [k, dh] per k-block, with a ones column at dh (denominator)
        v_sb = vpool.tile([128, NKB, DH + 1], f16, tag="vsb")
        nc.vector.tensor_copy(out=v_sb[:, :, DH], in_=onesc)
        # stacked normalized ctx^T: rows 0-63 head 2p, 64-127 head 2p+1;
        # axis 1 is the pair index
        ctx_sb = cpool.tile([128, PAIRS, T], f16, tag="ctx3", name="ctx3")

        # ---------------- helpers for interleaved emission ----------------
        pending_ex = {}

        def emit_stexp(qb, p, kb):
            q0 = QBW * qb
            kb_off = max(0, 128 * kb - q0)
            st = stp.tile([128, 1024], f32, tag="st", name="st")
            for h in range(2):
                o = 512 * h
                nc.tensor.matmul(
                    st[:, o + kb_off:o + 512],
                    lhsT=(kt_a[:, 128 * kb:128 * kb + 128] if h == 0 else
                          kv_sb[64:128, 128 * kb:128 * kb + 128]),
                    rhs=qt_sb[p][64 * h:64 * h + 64,
                                 q0 + kb_off:q0 + QBW],
                    start=True, stop=True, tile_position=(64 * h, 0))
            ex = epool.tile([128, 1024], f16, tag="ex", name="ex")
            if kb_off == 0:
                nc.scalar.activation(out=ex, in_=st, func=Exp)
            else:
                # one 3D-AP exp covering both heads' live columns
                st3 = st.rearrange("p (h q) -> p h q", h=2)
                ex3 = ex.rearrange("p (h q) -> p h q", h=2)
                nc.scalar.activation(
                    out=ex3[:, :, kb_off:512],
                    in_=st3[:, :, kb_off:512], func=Exp)
            if 128 * kb >= q0:  # diagonal block: causal mask (both heads
                # in one 3D-AP multiply on the idle Pool engine; mask
                # broadcast along the head dim)
                ex3m = ex.rearrange("p (h q) -> p h q", h=2)
                nc.gpsimd.tensor_mul(
                    ex3m[:, :, kb_off:kb_off + 128],
                    ex3m[:, :, kb_off:kb_off + 128],
                    mask.rearrange("p (h w) -> p h w", h=1).to_broadcast(
                        [128, 2, 128]))
            return ex

        # ---------------- Phase A: projections (t-quarters) ----------------
        with tc.tile_pool(name="pa_ps", bufs=1, space="PSUM") as pa:
            for tq in range(NTQ):
                t0 = TQW * tq
                qt_ps = [pa.tile([128, TQW], f32, tag=f"qtps{m}",
                                 name=f"qtps{m}") for m in range(2)]
                kv_ps = pa.tile([128, TQW], f32, tag="kvps")
                for grp in range(4):
                    if tq == 0 and grp == 0:
                        nc.sync.dma_start(out=w_sb[:, 0:4], in_=wpt_r[:, 0:4])
                    if tq == 0 and grp == 1:
                        nc.sync.dma_start(out=w_sb[:, 4:16],
                                          in_=wpt_r[:, 4:16])
                    xc = xpool.tile([128, 4, TQW], f16, tag="xc")
                    nc.sync.dma_start(
                        out=xc,
                        in_=xt.rearrange("(c p) n -> p c n", p=128)[
                            :, 4 * grp:4 * grp + 4, t0:t0 + TQW])
                    for ci4 in range(4):
                        ci = 4 * grp + ci4
                        for m in range(2):
                            nc.tensor.matmul(
                                qt_ps[m],
                                lhsT=w_sb[:, ci, 128 * m:128 * m + 128],
                                rhs=xc[:, ci4],
                                start=(ci == 0), stop=(ci == NCH - 1))
                        nc.tensor.matmul(
                            kv_ps, lhsT=w_sb[:, ci, 256:384], rhs=xc[:, ci4],
                            start=(ci == 0), stop=(ci == NCH - 1))
                for m in range(2):
                    nc.vector.tensor_copy(out=qt_sb[m][:, t0:t0 + TQW],
                                          in_=qt_ps[m])
                nc.vector.tensor_copy(out=kv_sb[:, t0:t0 + TQW], in_=kv_ps)
                # K^T duplicate for this quarter (cross-partition SBUF DMA)
                nc.sync.dma_start(out=kt_a[:, t0:t0 + TQW],
                                  in_=kv_sb[64:128, t0:t0 + TQW])
                # V natural via PE transpose of this quarter's V^T blocks
                for c in range(4 * tq, 4 * tq + 4):
                    tp = pa.tile([128, 64], f16, tag="vtr", name="vtr")
                    nc.tensor.transpose(
                        tp, in_=kv_sb[0:64, 128 * c:128 * c + 128],
                        identity=ident[0:64, 0:64])
                    nc.vector.tensor_copy(out=v_sb[:, c, 0:DH], in_=tp)
                # pre-emit attention ST/exp (runs in A's PE/ACT gaps)
                if tq == 0:
                    for p in range(PAIRS):
                        for kb in range(4):
                            pending_ex[(0, p, kb)] = emit_stexp(0, p, kb)
                elif tq == 1:
                    for kb in range(4):
                        pending_ex[(1, 0, kb)] = emit_stexp(1, 0, kb)
                elif tq == 2:
                    for kb in range(4):
                        pending_ex[(1, 1, kb)] = emit_stexp(1, 1, kb)

        # o_proj weights (first needed by phase C)
        nc.sync.dma_start(out=opj_sb[:, 0], in_=opj_r[:, 0])
        nc.sync.dma_start(out=opj_sb[:, 1], in_=opj_r[:, 1])

        # ---------------- Phase B (attention) + C (o_proj) ----------------
        with tc.tile_pool(name="ctx_ps", bufs=2, space="PSUM") as cxp, \
             tc.tile_pool(name="oc_ps", bufs=2, space="PSUM") as ocp:
            # C-unit state: emit o_proj tiles of the previous qb in drips
            cstate = {"units": [], "osb": None, "tt": -1, "tail": False}

            def emit_cunit():
                if not cstate["units"]:
                    return
                tt, jn = cstate["units"].pop(0)
                if cstate["tt"] != tt:
                    cstate["osb"] = opool.tile([128, D], f16, tag="osb",
                                               name="osb")
                    cstate["tt"] = tt
                osb = cstate["osb"]
                oc = ocp.tile([128, 512], f32, tag="oc", name="oc")
                for p in range(PAIRS):
                    nc.tensor.matmul(
                        oc,
                        lhsT=ctx_sb[:, p, 128 * tt:128 * tt + 128],
                        rhs=opj_sb[:, p, 512 * jn:512 * jn + 512],
                        start=(p == 0), stop=(p == PAIRS - 1))
                # PSUM evacuation: DVE while ACT is busy with exps; in the
                # final drain (no exps left) alternate DVE and ACT (Copy
                # shares the Exp activation table, so no table reloads)
                if cstate["tail"] and (tt + jn) % 2 == 1:
                    nc.scalar.activation(
                        out=osb[:, 512 * jn:512 * jn + 512], in_=oc,
                        func=Copy)
                else:
                    nc.vector.tensor_copy(
                        out=osb[:, 512 * jn:512 * jn + 512], in_=oc)
                if jn == 3:  # whole row block staged -> one DMA (Pool SWDGE)
                    nc.gpsimd.dma_start(
                        out=out[128 * tt:128 * tt + 128, :], in_=osb)

            # per-head AV passes: ctx tiles are one PSUM bank each, so two
            # can be live (bufs=2) and each head's normalize overlaps the
            # next head's AV pass; o_proj drips pack into the PE-light
            # second pass of each pair
            order = [(qb, p) for qb in range(NQB) for p in range(PAIRS)]
            for idx, (qb, p) in enumerate(order):
                q0 = QBW * qb
                nkb = 4 * qb + 4
                for h in range(2):
                    o = 512 * h
                    ctx = cxp.tile([DH + 1, 512], f32, tag="ctx")
                    for kb in range(nkb):
                        if h == 0:
                            # software-pipeline: ST/exp one block ahead of
                            # the AV consumer (both heads' logits at once)
                            for la in (kb, kb + 1):
                                if (la < nkb and
                                        (qb, p, la) not in pending_ex):
                                    pending_ex[(qb, p, la)] = \
                                        emit_stexp(qb, p, la)
                            ex = pending_ex[(qb, p, kb)]
                        else:
                            ex = pending_ex.pop((qb, p, kb))
                        # on diagonal blocks only cols >= kb_off are live
                        n0 = max(0, 128 * kb - q0)
                        nc.tensor.matmul(
                            ctx[:, n0:512],
                            lhsT=v_sb[:, kb, :],
                            rhs=ex[:, o + n0:o + 512],
                            start=(kb == 0), stop=(kb == nkb - 1))
                        if h == 1:
                            emit_cunit()
                    if h == 1 and idx + 1 < len(order):
                        # prefetch the next pair's first ST/exp blocks
                        nqb, npr = order[idx + 1]
                        for kb in range(min(2, 4 * nqb + 4)):
                            if (nqb, npr, kb) not in pending_ex:
                                pending_ex[(nqb, npr, kb)] = \
                                    emit_stexp(nqb, npr, kb)
                    # normalize by the softmax denominator (PSUM row 64):
                    # row -> SBUF (f16), replicate down 64 partitions with
                    # a K=1 matmul, then reciprocal and scale
                    densr = rpool.tile([65, 512], f16, tag="densr")
                    nc.vector.tensor_copy(
                        out=densr[64:65, :], in_=ctx[64:65, :])
                    repl_ps = ocp.tile([64, 512], f32, tag="oc",
                                       name="replps")
                    nc.tensor.matmul(
                        repl_ps, lhsT=onesr[64:65, 0:64],
                        rhs=densr[64:65, :],
                        start=True, stop=True, tile_position=(64, 0))
                    repl = rpool.tile([64, 512], f32, tag="repl")
                    nc.vector.reciprocal(out=repl, in_=repl_ps)
                    if h == 0:
                        nc.vector.tensor_mul(
                            ctx_sb[0:64, p, q0:q0 + QBW],
                            ctx[0:64, :], repl)
                    else:
                        stg = spool.tile([64, 512], f16, tag="stg",
                                         name="stg")
                        nc.vector.tensor_mul(stg, ctx[0:64, :], repl)
                        # head 2p+1 -> partitions 64-127 (cross-partition
                        # DMA)
                        nc.sync.dma_start(
                            out=ctx_sb[64:128, p, q0:q0 + QBW], in_=stg)
                if p == PAIRS - 1:
                    # queue this qb's o_proj tiles; drain qb-1 leftovers now
                    while cstate["units"]:
                        emit_cunit()
                    cstate["units"] = [(tt, jn)
                                       for tt in range(4 * qb, 4 * qb + 4)
                                       for jn in range(4)]
            cstate["tail"] = True
            while cstate["units"]:
                emit_cunit()

    nc.compile()
    return nc


def _get_nc():
    global _NC
    if _NC is None:
        _NC = build_nc()
    return _NC


def make_in_maps(x, q_proj, k_proj, v_proj, o_proj):
    x = np.asarray(x, np.float32).reshape(T, D)
    q_proj = np.asarray(q_proj, np.float32)
    k_proj = np.asarray(k_proj, np.float32)
    v_proj = np.asarray(v_proj, np.float32)
    o_proj = np.asarray(o_proj, np.float32)

    xt = np.ascontiguousarray(x.T).astype(np.float16)  # [D, T]
    scale = 1.0 / np.sqrt(np.float32(DH))
    maps = []
    for c in range(NCORES):
        qs = slice(4 * DH * c, 4 * DH * (c + 1))     # 256 q rows
        ks = slice(DH * c, DH * (c + 1))             # 64 kv rows
        m = {
            "xt": xt,
            # [q^T | v^T | k^T]: 384 columns
            "wpt": np.ascontiguousarray(np.concatenate(
                [q_proj[qs, :] * scale, v_proj[ks, :], k_proj[ks, :]],
                axis=0).T).astype(np.float16),
            "opj": np.ascontiguousarray(o_proj[qs, :]).astype(np.float16),
        }
        maps.append(m)
    return maps


def kernel(**inputs):
    global LAST_RESULT
    from concourse.bass_utils import run_bass_kernel_spmd
    nc = _get_nc()
    maps = make_in_maps(inputs["x"], inputs["q_proj"], inputs["k_proj"],
                        inputs["v_proj"], inputs["o_proj"])
    res = run_bass_kernel_spmd(
        nc, maps, list(range(NCORES)),
        trace=bool(int(os.environ.get("BASS_KERNEL_TRACE", "0"))))
    LAST_RESULT = res
    acc = np.zeros((T, D), np.float64)
    for c in range(NCORES):
        acc += res.results[c]["out"].astype(np.float64)
    return acc.astype(np.float32).reshape(1, T, D)


# revision 31
# speedup vs baseline: 1.8714x; 1.4610x over previous
"""Trainium2 Bass kernel: GQA causal self-attention block (B=1, T=2048, D=2048,
32 q-heads / 8 kv-heads, head_dim 64) with q/k/v/o projections.

Sharding: head-parallel (tensor parallel) across 8 NeuronCores.
Core c owns q-heads 4c..4c+3 and kv-head c:
  - computes Q^T/K^T (transposed, head-dim on partitions) and V (natural) for
    its heads from a host-pre-transposed x^T,
  - blockwise causal softmax(QK^T)V in a k-major layout (denominator obtained
    free via a ones-column appended to V),
  - a partial output projection out_c = ctx_c^T @ o_proj[rows_c, :].
The host sums the 8 partial outputs (the tensor-parallel reduction).

All activations/weights are fp16 (full PE rate at any moving size, so causal
blocks are trimmed to live columns; PSUM accumulation stays fp32). DMAs are
batched into few large instructions; output staging copies alternate between
the DVE and Pool engines and output DMAs ride the Pool SWDGE queue, which
avoids the shared HWDGE serialization.
"""

import os
import numpy as np

T = 2048
D = 2048
HQ, HKV = 32, 8
DH = 64
NCORES = 8
PAIRS = 2                 # 2 head-pairs per core (4 q heads)
NCH = D // 128            # 16 contraction chunks for projections
NTQ = 4                   # t-quarters in projection phase
TQW = T // NTQ            # 512
NQB = 4                   # q blocks of 512
QBW = 512
NKB = T // 128            # 16 k blocks of 128
WCOLS = 4 * DH + 2 * DH   # 384: [q^T (256) | v^T (64) | k^T (64)]

_NC = None
LAST_RESULT = None


def build_nc():
    import concourse.tile as tile
    from concourse import bacc, mybir
    from concourse.masks import make_identity, make_upper_triangular

    f16 = mybir.dt.float16
    f32 = mybir.dt.float32
    Exp = mybir.ActivationFunctionType.Exp
    Copy = mybir.ActivationFunctionType.Copy

    nc = bacc.Bacc("TRN2", target_bir_lowering=False, debug=False,
                   num_devices=NCORES)

    xt = nc.dram_tensor("xt", [D, T], f16, kind="ExternalInput").ap()
    wpt = nc.dram_tensor("wpt", [D, WCOLS], f16, kind="ExternalInput").ap()
    opj = nc.dram_tensor("opj", [4 * DH, D], f16, kind="ExternalInput").ap()
    out = nc.dram_tensor("out", [T, D], f16, kind="ExternalOutput").ap()

    from contextlib import ExitStack
    with tile.TileContext(nc) as tc, ExitStack() as ctx:
        consts = ctx.enter_context(tc.tile_pool(name="consts", bufs=1))
        wpool = ctx.enter_context(tc.tile_pool(name="weights", bufs=1))
        qtp = ctx.enter_context(tc.tile_pool(name="qt", bufs=1))
        ktp = ctx.enter_context(tc.tile_pool(name="kt", bufs=1))
        vpool = ctx.enter_context(tc.tile_pool(name="v", bufs=1))
        xpool = ctx.enter_context(tc.tile_pool(name="xchunk", bufs=4))
        epool = ctx.enter_context(tc.tile_pool(name="exps", bufs=18))
        cpool = ctx.enter_context(tc.tile_pool(name="ctxsb", bufs=1))
        spool = ctx.enter_context(tc.tile_pool(name="stage", bufs=2))
        opool = ctx.enter_context(tc.tile_pool(name="outsb", bufs=3))
        rpool = ctx.enter_context(tc.tile_pool(name="recip", bufs=2))
        # attention ST psum at stack bottom (banks 0-3) so phase B's ST/exp
        # can overlap phase A (whose psum lives in banks 4-7)
        stp = ctx.enter_context(tc.tile_pool(name="st_ps", bufs=2,
                                             space="PSUM"))

        # constants (built in f32 -- memset/affine_select write f32 -- then
        # converted to f16 via tensor_copy)
        identf = consts.tile([128, 128], f32, tag="identf")
        make_identity(nc, identf)
        ident = consts.tile([128, 128], f16, tag="ident")
        nc.vector.tensor_copy(out=ident, in_=identf)
        # mask[i, j] = 1.0 if i <= j else 0  (keep k_row <= q_col)
        maskf = consts.tile([128, 128], f32, tag="maskf")
        make_upper_triangular(nc, maskf, val=1.0, diag=True)
        mask = consts.tile([128, 128], f16, tag="mask")
        nc.vector.tensor_copy(out=mask, in_=maskf)
        onescf = consts.tile([128, NKB], f32, tag="onescf")
        nc.vector.memset(onescf, 1.0)
        onesc = consts.tile([128, NKB], f16, tag="onesc")
        nc.vector.tensor_copy(out=onesc, in_=onescf)
        onesrf = consts.tile([65, 64], f32, tag="onesrf")
        nc.vector.memset(onesrf, 1.0)
        onesr = consts.tile([65, 64], f16, tag="onesr")
        nc.vector.tensor_copy(out=onesr, in_=onesrf)

        # weights -> SBUF (two DMAs; first half unblocks the first matmuls)
        wpt_r = wpt.rearrange("(c p) n -> p c n", p=128)
        w_sb = wpool.tile([128, NCH, WCOLS], f16, tag="w")
        opj_r = opj.rearrange("(p r) j -> r p j", p=2)
        opj_sb = wpool.tile([128, 2, D], f16, tag="opj")

        # activation storage
        # qt_sb[p]: rows 0-63 = head 2p (Q^T), rows 64-127 = head 2p+1
        qt_sb = [qtp.tile([128, T], f16, tag=f"qt{p}", name=f"qt{p}")
                 for p in range(PAIRS)]
        # kv_sb: rows 0-63 = V^T, rows 64-127 = K^T
        kv_sb = ktp.tile([128, T], f16, tag="kv")
        # K^T copy on partitions 0-63 (for the tile_position (0,0) ST matmul)
        kt_a = ktp.tile([64, T], f16, tag="kta")
        # V natural [k, dh] per k-block, with a ones column at dh (denominator)
        v_sb = vpool.tile([128, NKB, DH + 1], f16, tag="vsb")
        nc.vector.tensor_copy(out=v_sb[:, :, DH], in_=onesc)
        # stacked normalized ctx^T: rows 0-63 head 2p, 64-127 head 2p+1;
        # axis 1 is the pair index
        ctx_sb = cpool.tile([128, PAIRS, T], f16, tag="ctx3", name="ctx3")

        # ---------------- helpers for interleaved emission ----------------
        pending_ex = {}

        def emit_stexp(qb, p, kb):
            q0 = QBW * qb
            kb_off = max(0, 128 * kb - q0)
            st = stp.tile([128, 1024], f32, tag="st", name="st")
            for h in range(2):
                o = 512 * h
                nc.tensor.matmul(
                    st[:, o + kb_off:o + 512],
                    lhsT=(kt_a[:, 128 * kb:128 * kb + 128] if h == 0 else
                          kv_sb[64:128, 128 * kb:128 * kb + 128]),
                    rhs=qt_sb[p][64 * h:64 * h + 64,
                                 q0 + kb_off:q0 + QBW],
                    start=True, stop=True, tile_position=(64 * h, 0))
            ex = epool.tile([128, 1024], f16, tag="ex", name="ex")
            if kb_off == 0:
                nc.scalar.activation(out=ex, in_=st, func=Exp)
            else:
                # one 3D-AP exp covering both heads' live columns
                st3 = st.rearrange("p (h q) -> p h q", h=2)
                ex3 = ex.rearrange("p (h q) -> p h q", h=2)
                nc.scalar.activation(
                    out=ex3[:, :, kb_off:512],
                    in_=st3[:, :, kb_off:512], func=Exp)
            if 128 * kb >= q0:  # diagonal block: causal mask (both heads
                # in one 3D-AP multiply on the idle Pool engine; mask
                # broadcast along the head dim)
                ex3m = ex.rearrange("p (h q) -> p h q", h=2)
                nc.gpsimd.tensor_mul(
                    ex3m[:, :, kb_off:kb_off + 128],
                    ex3m[:, :, kb_off:kb_off + 128],
                    mask.rearrange("p (h w) -> p h w", h=1).to_broadcast(
                        [128, 2, 128]))
            return ex

        # ---------------- Phase A: projections (t-quarters) ----------------
        with tc.tile_pool(name="pa_ps", bufs=1, space="PSUM") as pa:
            for tq in range(NTQ):
                t0 = TQW * tq
                qt_ps = [pa.tile([128, TQW], f32, tag=f"qtps{m}",
                                 name=f"qtps{m}") for m in range(2)]
                kv_ps = pa.tile([128, TQW], f32, tag="kvps")
                # tq0 loads split finer so the first matmul starts early;
                # weight pieces interleave with the x chunks that need them
                groups = [2, 2, 4, 4, 4] if tq == 0 else [4, 4, 4, 4]
                ci = 0
                for gi, g in enumerate(groups):
                    if tq == 0:
                        nc.sync.dma_start(out=w_sb[:, ci:ci + g],
                                          in_=wpt_r[:, ci:ci + g])
                    xc = xpool.tile([128, 4, TQW], f16, tag="xc")
                    nc.sync.dma_start(
                        out=xc[:, 0:g],
                        in_=xt.rearrange("(c p) n -> p c n", p=128)[
                            :, ci:ci + g, t0:t0 + TQW])
                    for cig in range(g):
                        for m in range(2):
                            nc.tensor.matmul(
                                qt_ps[m],
                                lhsT=w_sb[:, ci, 128 * m:128 * m + 128],
                                rhs=xc[:, cig],
                                start=(ci == 0), stop=(ci == NCH - 1))
                        nc.tensor.matmul(
                            kv_ps, lhsT=w_sb[:, ci, 256:384], rhs=xc[:, cig],
                            start=(ci == 0), stop=(ci == NCH - 1))
                        ci += 1
                for m in range(2):
                    nc.vector.tensor_copy(out=qt_sb[m][:, t0:t0 + TQW],
                                          in_=qt_ps[m])
                nc.vector.tensor_copy(out=kv_sb[:, t0:t0 + TQW], in_=kv_ps)
                # K^T duplicate for this quarter (cross-partition SBUF DMA)
                nc.sync.dma_start(out=kt_a[:, t0:t0 + TQW],
                                  in_=kv_sb[64:128, t0:t0 + TQW])
                # V natural via PE transpose of this quarter's V^T blocks
                for c in range(4 * tq, 4 * tq + 4):
                    tp = pa.tile([128, 64], f16, tag="vtr", name="vtr")
                    nc.tensor.transpose(
                        tp, in_=kv_sb[0:64, 128 * c:128 * c + 128],
                        identity=ident[0:64, 0:64])
                    nc.vector.tensor_copy(out=v_sb[:, c, 0:DH], in_=tp)
                # pre-emit attention ST/exp (runs in A's PE/ACT gaps)
                if tq == 0:
                    for p in range(PAIRS):
                        for kb in range(4):
                            pending_ex[(0, p, kb)] = emit_stexp(0, p, kb)
                elif tq == 1:
                    for kb in range(4):
                        pending_ex[(1, 0, kb)] = emit_stexp(1, 0, kb)
                elif tq == 2:
                    for kb in range(4):
                        pending_ex[(1, 1, kb)] = emit_stexp(1, 1, kb)

        # o_proj weights (first needed by phase C)
        nc.sync.dma_start(out=opj_sb[:, 0], in_=opj_r[:, 0])
        nc.sync.dma_start(out=opj_sb[:, 1], in_=opj_r[:, 1])

        # ---------------- Phase B (attention) + C (o_proj) ----------------
        with tc.tile_pool(name="ctx_ps", bufs=2, space="PSUM") as cxp, \
             tc.tile_pool(name="oc_ps", bufs=2, space="PSUM") as ocp:
            # C-unit state: emit o_proj tiles of the previous qb in drips
            cstate = {"units": [], "osb": None, "tt": -1, "tail": False}

            def emit_cunit():
                if not cstate["units"]:
                    return
                tt, jn = cstate["units"].pop(0)
                if cstate["tt"] != tt:
                    cstate["osb"] = opool.tile([128, D], f16, tag="osb",
                                               name="osb")
                    cstate["tt"] = tt
                osb = cstate["osb"]
                oc = ocp.tile([128, 512], f32, tag="oc", name="oc")
                for p in range(PAIRS):
                    nc.tensor.matmul(
                        oc,
                        lhsT=ctx_sb[:, p, 128 * tt:128 * tt + 128],
                        rhs=opj_sb[:, p, 512 * jn:512 * jn + 512],
                        start=(p == 0), stop=(p == PAIRS - 1))
                # PSUM evacuation: DVE while ACT is busy with exps; in the
                # final drain (no exps left) alternate DVE and ACT (Copy
                # shares the Exp activation table, so no table reloads)
                if cstate["tail"] and (tt + jn) % 2 == 1:
                    nc.scalar.activation(
                        out=osb[:, 512 * jn:512 * jn + 512], in_=oc,
                        func=Copy)
                else:
                    nc.vector.tensor_copy(
                        out=osb[:, 512 * jn:512 * jn + 512], in_=oc)
                if cstate["tail"] and jn == 1:
                    # tail: half-row DMA so the final transfer starts sooner
                    nc.sync.dma_start(
                        out=out[128 * tt:128 * tt + 128, 0:1024],
                        in_=osb[:, 0:1024])
                elif cstate["tail"] and jn == 3:
                    nc.sync.dma_start(
                        out=out[128 * tt:128 * tt + 128, 1024:2048],
                        in_=osb[:, 1024:2048])
                elif jn == 3:  # whole row block staged -> one DMA
                    nc.sync.dma_start(
                        out=out[128 * tt:128 * tt + 128, :], in_=osb)

            # per-head AV passes: ctx tiles are one PSUM bank each, so two
            # can be live (bufs=2) and each head's normalize overlaps the
            # next head's AV pass; o_proj drips pack into the PE-light
            # second pass of each pair
            order = [(qb, p) for qb in range(NQB) for p in range(PAIRS)]
            for idx, (qb, p) in enumerate(order):
                q0 = QBW * qb
                nkb = 4 * qb + 4
                # head 2p+1 (h=1) first: its staged cross-partition DMA
                # issues in the first pass and lands under the second
                for hi, h in enumerate((1, 0)):
                    o = 512 * h
                    ctx = cxp.tile([DH + 1, 512], f32, tag="ctx")
                    for kb in range(nkb):
                        if hi == 0:
                            # software-pipeline: ST/exp one block ahead of
                            # the AV consumer (both heads' logits at once)
                            for la in (kb, kb + 1):
                                if (la < nkb and
                                        (qb, p, la) not in pending_ex):
                                    pending_ex[(qb, p, la)] = \
                                        emit_stexp(qb, p, la)
                            ex = pending_ex[(qb, p, kb)]
                        else:
                            ex = pending_ex.pop((qb, p, kb))
                        # on diagonal blocks only cols >= kb_off are live
                        n0 = max(0, 128 * kb - q0)
                        nc.tensor.matmul(
                            ctx[:, n0:512],
                            lhsT=v_sb[:, kb, :],
                            rhs=ex[:, o + n0:o + 512],
                            start=(kb == 0), stop=(kb == nkb - 1))
                        if hi == 1:
                            emit_cunit()
                    if hi == 1 and idx + 1 < len(order):
                        # prefetch the next pair's first ST/exp blocks
                        nqb, npr = order[idx + 1]
                        for kb in range(min(2, 4 * nqb + 4)):
                            if (nqb, npr, kb) not in pending_ex:
                                pending_ex[(nqb, npr, kb)] = \
                                    emit_stexp(nqb, npr, kb)
                    # normalize by the softmax denominator (PSUM row 64):
                    # row -> SBUF (f16), replicate down 64 partitions with
                    # a K=1 matmul, then reciprocal and scale
                    densr = rpool.tile([65, 512], f16, tag="densr")
                    nc.vector.tensor_copy(
                        out=densr[64:65, :], in_=ctx[64:65, :])
                    repl_ps = ocp.tile([64, 512], f32, tag="oc",
                                       name="replps")
                    nc.tensor.matmul(
                        repl_ps, lhsT=onesr[64:65, 0:64],
                        rhs=densr[64:65, :],
                        start=True, stop=True, tile_position=(64, 0))
                    repl = rpool.tile([64, 512], f32, tag="repl")
                    nc.vector.reciprocal(out=repl, in_=repl_ps)
                    if h == 0:
                        nc.vector.tensor_mul(
                            ctx_sb[0:64, p, q0:q0 + QBW],
                            ctx[0:64, :], repl)
                    else:
                        stg = spool.tile([64, 512], f16, tag="stg",
                                         name="stg")
                        nc.vector.tensor_mul(stg, ctx[0:64, :], repl)
                        # head 2p+1 -> partitions 64-127 (cross-partition
                        # DMA)
                        nc.sync.dma_start(
                            out=ctx_sb[64:128, p, q0:q0 + QBW], in_=stg)
                if p == PAIRS - 1:
                    # queue this qb's o_proj tiles; drain qb-1 leftovers now
                    while cstate["units"]:
                        emit_cunit()
                    cstate["units"] = [(tt, jn)
                                       for tt in range(4 * qb, 4 * qb + 4)
                                       for jn in range(4)]
            cstate["tail"] = True
            while cstate["units"]:
                emit_cunit()

    nc.compile()
    return nc


def _get_nc():
    global _NC
    if _NC is None:
        _NC = build_nc()
    return _NC


def make_in_maps(x, q_proj, k_proj, v_proj, o_proj):
    x = np.asarray(x, np.float32).reshape(T, D)
    q_proj = np.asarray(q_proj, np.float32)
    k_proj = np.asarray(k_proj, np.float32)
    v_proj = np.asarray(v_proj, np.float32)
    o_proj = np.asarray(o_proj, np.float32)

    xt = np.ascontiguousarray(x.T).astype(np.float16)  # [D, T]
    scale = 1.0 / np.sqrt(np.float32(DH))
    maps = []
    for c in range(NCORES):
        qs = slice(4 * DH * c, 4 * DH * (c + 1))     # 256 q rows
        ks = slice(DH * c, DH * (c + 1))             # 64 kv rows
        m = {
            "xt": xt,
            # [q^T | v^T | k^T]: 384 columns
            "wpt": np.ascontiguousarray(np.concatenate(
                [q_proj[qs, :] * scale, v_proj[ks, :], k_proj[ks, :]],
                axis=0).T).astype(np.float16),
            "opj": np.ascontiguousarray(o_proj[qs, :]).astype(np.float16),
        }
        maps.append(m)
    return maps


def kernel(**inputs):
    global LAST_RESULT
    from concourse.bass_utils import run_bass_kernel_spmd
    nc = _get_nc()
    maps = make_in_maps(inputs["x"], inputs["q_proj"], inputs["k_proj"],
                        inputs["v_proj"], inputs["o_proj"])
    res = run_bass_kernel_spmd(
        nc, maps, list(range(NCORES)),
        trace=bool(int(os.environ.get("BASS_KERNEL_TRACE", "0"))))
    LAST_RESULT = res
    acc = np.zeros((T, D), np.float64)
    for c in range(NCORES):
        acc += res.results[c]["out"].astype(np.float64)
    return acc.astype(np.float32).reshape(1, T, D)
